# revision 1
# baseline (speedup 1.0000x reference)
"""DeepSeekMoE layer on 8 Trainium2 NeuronCores.

Strategy (expert-parallel):
  - Host: RMSNorm + router matmul + top-k + per-expert token gather
    (routing is tiny: 2048x1024 @ 1024x64). Tokens for each routed
    expert are gathered into per-expert slots and bin-packed onto the
    8 cores (rank-grouped so slot j holds similarly-sized experts on
    every core; per-slot capacities are compile-time constants rounded
    to 16).
  - Device (SPMD, one program on cores 0-7): for each expert slot,
    stream W1/W2 from HBM once and run the FFN on the gathered token
    batch entirely transposed ([D, tokens] layout) so every matmul
    contracts along partitions:
        h^T = W1^T x^T (+b1); g^T = silu(Wg^T h^T) * h^T;
        o^T = W2^T g^T (+b2)
    Default precision "fp8": e3m4 weights (stored x256, unscaled on the
    PSUM->SBUF copy) and e3m4 gathered tokens, bf16 intermediate
    activations, fp32 PSUM accumulate (~2e-3 final rel err, ~14MB HBM
    traffic per core). Weight loads issue on the sync HWDGE ring in
    0.5MB chunks (bigger chunks measurably steal SBUF ports from the
    PE), token loads on the scalar ring; output stores also ride the
    sync ring but are emitted two sections late so a store (which waits
    on compute) never head-of-line-blocks weight prefetch. Section 0
    loads W1 as 4 m-tiles and tokens as 8 k-tiles to cut the cold-start
    head; the final section stores per-chunk to cut the tail. PSUM
    tiles hold pairs of m-tiles so PSUM->SBUF moves are single fat DVE
    ops when biases are zero.
  - Host: scatter-add weighted expert outputs + shared + residual.

Self-contained: shapes hardcoded for B=2, S=1024, D=1024, H=512,
E_R=64, K=6, E_S=2.
"""

import numpy as np
from contextlib import ExitStack

B, S, D, H, E_R, K, E_S = 2, 1024, 1024, 512, 64, 6, 2
T = B * S
EPS = 1.1920929e-07

PREC = "fp8"         # "fp8" (e3m4 weights+tokens, ~2e-3 rel err),
                     # "bf16" (~3.6e-4), or "f32r" (~2e-5, 2x traffic)
WSCALE = 256.0       # fp8: weights are stored as w*WSCALE in e3m4 (max |w|
INV_WSCALE = 1.0 / WSCALE  # ~1/32 -> 8.0, within e3m4's +-15.5 normal range)
# the shared 512-token section runs DoubleRow (2 fp8 weights/PE cell, ~1.7x)
# which requires e4m3 operands; power-of-2 scales place values in e4m3 range
W4SC = 4096.0        # shared weights stored as w*4096 (max ~128 < 240)
T4SC = 16.0          # shared tokens stored as x*16
H4SC = 32.0          # shared h/g intermediates stored as h*32
INV_H4 = H4SC / (W4SC * T4SC)   # PSUM -> h tile
INV_G4 = 1.0 / (W4SC * H4SC)    # PSUM -> silu input / output tile
N_SLOTS = 8          # routed expert slots per core
SH_TOK = 512         # shared-expert tokens per core
KD = D // 128        # 8 k-tiles for the D contraction
KH = H // 128        # 4 k-tiles for the H contraction

_PROG_CACHE = {}


def _np_wdt():
    import ml_dtypes
    if PREC == "bf16":
        return ml_dtypes.bfloat16
    if PREC == "fp8":
        return ml_dtypes.float8_e3m4
    return np.float32


def _np_adt():
    import ml_dtypes
    if PREC == "f32r":
        return np.float32
    return ml_dtypes.bfloat16


def _np_tdt():
    """Host dtype for token blocks (device moving operand of the W1 stage)."""
    import ml_dtypes
    if PREC == "fp8":
        return ml_dtypes.float8_e3m4
    return _np_adt()


def _wcast(w):
    """Cast a prearranged fp32 weight block to the device weight dtype,
    scaling into e3m4 range for fp8."""
    if PREC == "fp8":
        return np.clip(w * WSCALE, -15.5, 15.5).astype(_np_wdt())
    return w.astype(_np_wdt())


def _e4cast(w, scale):
    import ml_dtypes
    return np.clip(w.astype(np.float32) * scale, -240.0, 240.0).astype(
        ml_dtypes.float8_e4m3fn)


def _prearrange(w, ktiles, raw=False):
    """[K*128, N] -> [128, K*N] so each SBUF partition's row is one
    contiguous DRAM read."""
    n = w.shape[1]
    out = np.ascontiguousarray(
        w.reshape(ktiles, 128, n).transpose(1, 0, 2).reshape(128, ktiles * n)
    )
    return out if raw else _wcast(out)


def _prearrange_w1(w, raw=False):
    """[D, H] -> [128, KH*KD*128] m-major ([m][k][i] per partition) so W1 can
    load as 4 independent m-tiles."""
    out = np.ascontiguousarray(
        w.reshape(KD, 128, KH, 128).transpose(1, 2, 0, 3).reshape(128, KH * KD * 128)
    )
    return out if raw else _wcast(out)


def _build_program(caps, has_bias):
    """caps: per-section token capacities; the last entry is the shared
    512-token section, the rest are routed expert slots."""
    import concourse.tile as tile
    from concourse import bacc, mybir

    f32 = mybir.dt.float32
    if PREC == "fp8":
        wdt = mybir.dt.float8e3
    elif PREC == "f32r":
        wdt = mybir.dt.float32r
    else:
        wdt = mybir.dt.bfloat16
    adt = mybir.dt.float32r if PREC == "f32r" else mybir.dt.bfloat16
    tdt = mybir.dt.float8e3 if PREC == "fp8" else adt
    odt = f32 if PREC == "f32r" else mybir.dt.bfloat16
    inv = INV_WSCALE if PREC == "fp8" else 1.0
    AF = mybir.ActivationFunctionType
    ALU = mybir.AluOpType
    # the shared section uses DoubleRow e4m3 when fp8 and bias-free
    dr_shared = PREC == "fp8" and not has_bias
    e4 = mybir.dt.float8e4
    swdt = e4 if dr_shared else wdt
    stdt = e4 if dr_shared else tdt

    n_slots = len(caps) - 1
    offs = np.concatenate([[0], np.cumsum(caps)])
    sumcap = int(offs[-2])          # routed columns only
    sh_cap = caps[-1]

    nc = bacc.Bacc("TRN2", target_bir_lowering=False, debug=False)

    xgt = nc.dram_tensor("xgt", [128, KD * sumcap], tdt, kind="ExternalInput").ap()
    w1s = nc.dram_tensor("w1s", [n_slots, 128, KD * H], wdt, kind="ExternalInput").ap()
    w2s = nc.dram_tensor("w2s", [n_slots, 128, KH * D], wdt, kind="ExternalInput").ap()
    b1s = nc.dram_tensor("b1s", [128, n_slots * 4], f32, kind="ExternalInput").ap()
    b2s = nc.dram_tensor("b2s", [128, n_slots * 8], f32, kind="ExternalInput").ap()
    wg = nc.dram_tensor("wg", [128, KH * H], wdt, kind="ExternalInput").ap()
    if dr_shared:
        swg = nc.dram_tensor("swg", [128, KH, H], swdt, kind="ExternalInput").ap()
        tsht = nc.dram_tensor("tsht", [128, KD, sh_cap], stdt,
                              kind="ExternalInput").ap()
        sw1 = nc.dram_tensor("sw1", [128, KH * KD, 128], swdt,
                             kind="ExternalInput").ap()
        sw2 = nc.dram_tensor("sw2", [128, KH, D], swdt, kind="ExternalInput").ap()
    else:
        swg = nc.dram_tensor("swg", [128, KH * H], wdt, kind="ExternalInput").ap()
        tsht = nc.dram_tensor("tsht", [128, KD * sh_cap], tdt,
                              kind="ExternalInput").ap()
        sw1 = nc.dram_tensor("sw1", [128, KD * H], wdt, kind="ExternalInput").ap()
        sw2 = nc.dram_tensor("sw2", [128, KH * D], wdt, kind="ExternalInput").ap()
    sb1 = nc.dram_tensor("sb1", [128, 4], f32, kind="ExternalInput").ap()
    sb2 = nc.dram_tensor("sb2", [128, 8], f32, kind="ExternalInput").ap()
    rout = nc.dram_tensor("rout", [128, KD * sumcap], odt, kind="ExternalOutput").ap()
    sout = nc.dram_tensor("sout", [128, KD * sh_cap], odt, kind="ExternalOutput").ap()

    with tile.TileContext(nc) as tc:
        with ExitStack() as ctx:
            consts = ctx.enter_context(tc.tile_pool(name="consts", bufs=1))
            w1p = ctx.enter_context(tc.tile_pool(name="w1p", bufs=5))
            w2p = ctx.enter_context(tc.tile_pool(name="w2p", bufs=5))
            xgp = ctx.enter_context(tc.tile_pool(name="xgp", bufs=4))
            hp = ctx.enter_context(tc.tile_pool(name="hp", bufs=3))
            gpp = ctx.enter_context(tc.tile_pool(name="gpp", bufs=2))
            gp = ctx.enter_context(tc.tile_pool(name="gp", bufs=3))
            op = ctx.enter_context(tc.tile_pool(name="op", bufs=3))
            php = ctx.enter_context(tc.tile_pool(name="php", bufs=3, space="PSUM"))
            pgp = ctx.enter_context(tc.tile_pool(name="pgp", bufs=2, space="PSUM"))
            pop = ctx.enter_context(tc.tile_pool(name="pop", bufs=3, space="PSUM"))

            # biases ride the idle gpsimd ring so the sync ring's first
            # issue is section 0's W1; gate weights ride the scalar ring
            # right after the first token load (see after_xg below)
            b1_sb = consts.tile([128, n_slots * 4], f32, tag="b1")
            nc.gpsimd.dma_start(b1_sb[:], b1s[:])
            b2_sb = consts.tile([128, n_slots * 8], f32, tag="b2")
            nc.gpsimd.dma_start(b2_sb[:], b2s[:])
            sb1_sb = consts.tile([128, 4], f32, tag="sb1")
            nc.gpsimd.dma_start(sb1_sb[:], sb1[:])
            sb2_sb = consts.tile([128, 8], f32, tag="sb2")
            nc.gpsimd.dma_start(sb2_sb[:], sb2[:])
            wg_sb = consts.tile([128, KH * H], wdt, tag="wg")
            if dr_shared:
                swg_sb = consts.tile([128, KH, H], swdt, tag="swg")
            else:
                swg_sb = consts.tile([128, KH * H], wdt, tag="swg")

            # warm-up: dependency-free matmuls on (uninitialized) SBUF run
            # while the first weight DMAs stream in, releasing the PE HAM
            # clock throttle before real work arrives
            warm_src = consts.tile([128, 256], adt, tag="warmsrc", name="warmsrc")
            nc.vector.memset(warm_src[:], 0.0)
            warm_ps = php.tile([128, 256], f32, tag="ph", name="warmps")
            for _ in range(12):
                nc.tensor.matmul(
                    warm_ps[:], warm_src[:, 0:128], warm_src[:],
                    start=True, stop=True,
                )

            def section(cap, w1sb, w2_src, xg_src, out_dst, wgt, b1ap, b2ap,
                        after_xg=None, last=False, w1_tiles=None, xg_tiles=None,
                        after_loads=None, issue_next=None):
                """One expert FFN pass over `cap` tokens (transposed layout).
                W1 arrives preloaded (issued one section earlier on the sync
                ring, ahead of the previous section's W2, so the W1 stage
                never waits); issue_next emits the NEXT section's W1 load
                before this section's W2 load. PSUM tiles hold pairs of
                m-tiles (one bank) so PSUM->SBUF moves are single fat ops
                when biases are zero."""
                p2 = 2 if cap <= 256 else 1   # m-tiles per PSUM bank
                if xg_tiles is None:
                    xgsb = xgp.tile([128, KD * cap], tdt, tag="xg")
                    nc.scalar.dma_start(xgsb[:], xg_src)
                if after_xg is not None:
                    after_xg()
                if issue_next is not None:
                    issue_next()
                w2sb = w2p.tile([128, KH * D], wdt, tag="w2")
                nc.sync.dma_start(w2sb[:], w2_src)
                if after_loads is not None:
                    after_loads()

                def w1ap(m, k):
                    if w1_tiles is not None:
                        return w1_tiles[m][:, k * 128 : (k + 1) * 128]
                    return w1sb[:, m * KD * 128 + k * 128 : m * KD * 128 + (k + 1) * 128]

                def xgap(k):
                    if xg_tiles is not None:
                        return xg_tiles[k][:]
                    return xgsb[:, k * cap : (k + 1) * cap]

                hsb = hp.tile([128, KH * cap], adt, tag="h")
                for mp in range(KH // p2):
                    ph = php.tile([128, p2 * cap], f32, tag="ph")
                    for sub in range(p2):
                        m = p2 * mp + sub
                        for k in range(KD):
                            nc.tensor.matmul(
                                ph[:, sub * cap : (sub + 1) * cap],
                                w1ap(m, k),
                                xgap(k),
                                start=(k == 0),
                                stop=(k == KD - 1),
                            )
                    if has_bias:
                        for sub in range(p2):
                            m = p2 * mp + sub
                            nc.vector.tensor_scalar(
                                hsb[:, m * cap : (m + 1) * cap],
                                ph[:, sub * cap : (sub + 1) * cap],
                                inv, b1ap[:, m : m + 1],
                                ALU.mult, ALU.add,
                            )
                    elif inv != 1.0:
                        nc.vector.tensor_scalar_mul(
                            hsb[:, p2 * mp * cap : p2 * (mp + 1) * cap], ph[:], inv
                        )
                    else:
                        nc.vector.tensor_copy(
                            hsb[:, p2 * mp * cap : p2 * (mp + 1) * cap], ph[:]
                        )

                gsb = gp.tile([128, KH * cap], adt, tag="g")
                for mp in range(KH // p2):
                    pg = pgp.tile([128, p2 * cap], f32, tag="pg")
                    for sub in range(p2):
                        m = p2 * mp + sub
                        for k in range(KH):
                            nc.tensor.matmul(
                                pg[:, sub * cap : (sub + 1) * cap],
                                wgt[:, k * H + m * 128 : k * H + (m + 1) * 128],
                                hsb[:, k * cap : (k + 1) * cap],
                                start=(k == 0),
                                stop=(k == KH - 1),
                            )
                    gpre = gpp.tile([128, p2 * cap], adt, tag="gpre")
                    nc.scalar.activation(gpre[:], pg[:], AF.Silu, scale=inv)
                    nc.vector.tensor_mul(
                        gsb[:, p2 * mp * cap : p2 * (mp + 1) * cap],
                        gpre[:],
                        hsb[:, p2 * mp * cap : p2 * (mp + 1) * cap],
                    )

                osb = op.tile([128, KD * cap], odt, tag="o")
                for mp in range(KD // p2):
                    po = pop.tile([128, p2 * cap], f32, tag="po")
                    for sub in range(p2):
                        m = p2 * mp + sub
                        for k in range(KH):
                            nc.tensor.matmul(
                                po[:, sub * cap : (sub + 1) * cap],
                                w2sb[:, k * D + m * 128 : k * D + (m + 1) * 128],
                                gsb[:, k * cap : (k + 1) * cap],
                                start=(k == 0),
                                stop=(k == KH - 1),
                            )
                    if has_bias:
                        for sub in range(p2):
                            m = p2 * mp + sub
                            nc.vector.tensor_scalar(
                                osb[:, m * cap : (m + 1) * cap],
                                po[:, sub * cap : (sub + 1) * cap],
                                inv, b2ap[:, m : m + 1],
                                ALU.mult, ALU.add,
                            )
                    elif inv != 1.0:
                        nc.vector.tensor_scalar_mul(
                            osb[:, p2 * mp * cap : p2 * (mp + 1) * cap], po[:], inv
                        )
                    else:
                        nc.vector.tensor_copy(
                            osb[:, p2 * mp * cap : p2 * (mp + 1) * cap], po[:]
                        )
                    if last:
                        # final section: store chunks immediately so the
                        # kernel tail is one chunk, not 2MB
                        nc.sync.dma_start(
                            out_dst[:, p2 * mp * cap : p2 * (mp + 1) * cap],
                            osb[:, p2 * mp * cap : p2 * (mp + 1) * cap],
                        )
                if last:
                    return None
                def store():
                    nc.sync.dma_start(out_dst, osb[:])
                return store

            def section_dr_shared(w1sb, after_loads=None, issue_next=None):
                """Shared-expert section in DoubleRow mode: e4m3 operands,
                2 k-tiles per matmul at 2 fp8 weights per PE cell (~1.7x the
                bf16-rate stream at N=512). W1 arrives preloaded like the
                routed sections."""
                DR = mybir.MatmulPerfMode.DoubleRow
                cap = sh_cap
                xsb = xgp.tile([128, KD, cap], stdt, tag="xg4")
                nc.scalar.dma_start(xsb[:], tsht[:])
                if issue_next is not None:
                    issue_next()
                w2sb = w2p.tile([128, KH, D], swdt, tag="w24")
                nc.sync.dma_start(w2sb[:], sw2[:])
                if after_loads is not None:
                    after_loads()

                hsb = hp.tile([128, KH, cap], swdt, tag="h4")
                for m in range(KH):
                    ph = php.tile([128, cap], f32, tag="ph")
                    for kp in range(KD // 2):
                        nc.tensor.matmul(
                            ph[:],
                            w1sb[:, m * KD + 2 * kp : m * KD + 2 * kp + 2, :],
                            xsb[:, 2 * kp : 2 * kp + 2, :],
                            start=(kp == 0),
                            stop=(kp == KD // 2 - 1),
                            perf_mode=DR,
                        )
                    nc.vector.tensor_scalar_mul(hsb[:, m, :], ph[:], INV_H4)

                gsb = gp.tile([128, KH, cap], swdt, tag="g4")
                for m in range(KH):
                    pg = pgp.tile([128, cap], f32, tag="pg")
                    for kp in range(KH // 2):
                        nc.tensor.matmul(
                            pg[:],
                            swg_sb[:, 2 * kp : 2 * kp + 2, m * 128 : (m + 1) * 128],
                            hsb[:, 2 * kp : 2 * kp + 2, :],
                            start=(kp == 0),
                            stop=(kp == KH // 2 - 1),
                            perf_mode=DR,
                        )
                    gpre = gpp.tile([128, cap], adt, tag="gpre")
                    nc.scalar.activation(gpre[:], pg[:], AF.Silu, scale=INV_G4)
                    # hsb holds 32*h so the product lands at 32*g, already in
                    # e4m3 range for the W2 stage
                    nc.vector.tensor_mul(gsb[:, m, :], gpre[:], hsb[:, m, :])

                osb = op.tile([128, KD * cap], odt, tag="o")
                for m in range(KD):
                    po = pop.tile([128, cap], f32, tag="po")
                    for kp in range(KH // 2):
                        nc.tensor.matmul(
                            po[:],
                            w2sb[:, 2 * kp : 2 * kp + 2, m * 128 : (m + 1) * 128],
                            gsb[:, 2 * kp : 2 * kp + 2, :],
                            start=(kp == 0),
                            stop=(kp == KH // 2 - 1),
                            perf_mode=DR,
                        )
                    nc.vector.tensor_scalar_mul(
                        osb[:, m * cap : (m + 1) * cap], po[:], INV_G4
                    )

                def store():
                    nc.sync.dma_start(sout[:], osb[:])
                return store

            def load_wg():
                nc.scalar.dma_start(wg_sb[:], wg[:])

            def load_swg():
                nc.scalar.dma_start(swg_sb[:], swg[:])

            # section 0 fast start: W1 as 4 independent m-tiles and tokens as
            # 8 k-tiles so the first matmul waits for ~160KB, not 1MB
            c0 = caps[0]
            sec0_xg = [consts.tile([128, c0], tdt, tag=f"xh{k}", name=f"xh{k}") for k in range(KD)]
            nc.scalar.dma_start(sec0_xg[0][:], xgt[:, 0:c0])
            nc.scalar.dma_start(sec0_xg[1][:], xgt[:, c0 : 2 * c0])
            load_wg()
            for k in range(2, KD):
                nc.scalar.dma_start(sec0_xg[k][:], xgt[:, k * c0 : (k + 1) * c0])
            sec0_w1 = [consts.tile([128, KD * 128], wdt, tag=f"w1h{m}", name=f"w1h{m}") for m in range(KH)]
            for m in range(KH):
                nc.sync.dma_start(
                    sec0_w1[m][:], w1s[0][:, m * KD * 128 : (m + 1) * KD * 128]
                )

            pending = []

            def flush_store():
                if len(pending) >= 2:
                    pending.pop(0)()

            def flush_all():
                while pending:
                    pending.pop(0)()

            # the shared section is PE-heavy relative to its DMA bytes, so it
            # runs mid-stream as a prefetch catch-up window; the kernel ends
            # on a routed slot with eager per-chunk stores
            calls = [("r", j) for j in range(4)] + [("s", None)] + \
                    [("r", j) for j in range(4, n_slots)]

            def w1_loader(pos):
                kind, j = calls[pos]
                if kind == "s" and dr_shared:
                    tl = w1p.tile([128, KH * KD, 128], swdt, tag="w14")
                    nc.sync.dma_start(tl[:], sw1[:])
                elif kind == "s":
                    tl = w1p.tile([128, KD * H], wdt, tag="w1")
                    nc.sync.dma_start(tl[:], sw1[:])
                else:
                    tl = w1p.tile([128, KD * H], wdt, tag="w1")
                    nc.sync.dma_start(tl[:], w1s[j])
                return tl

            pre_w1 = [None] * len(calls)
            pre_w1[1] = w1_loader(1)   # behind sec0's m-tiles, ahead of W2(0)

            def make_issue_next(pos):
                def f():
                    if pos + 1 < len(calls) and pre_w1[pos + 1] is None:
                        pre_w1[pos + 1] = w1_loader(pos + 1)
                return f

            for pos, (kind, j) in enumerate(calls):
                is_last = pos == len(calls) - 1
                if kind == "r":
                    lo, hi = int(offs[j]) * KD, int(offs[j + 1]) * KD
                    st = section(
                        caps[j],
                        pre_w1[pos],
                        w2s[j],
                        xgt[:, lo:hi],
                        rout[:, lo:hi],
                        wg_sb,
                        b1_sb[:, j * 4 : (j + 1) * 4],
                        b2_sb[:, j * 8 : (j + 1) * 8],
                        after_xg=load_swg if j == 1 else None,
                        w1_tiles=sec0_w1 if j == 0 else None,
                        xg_tiles=sec0_xg if j == 0 else None,
                        last=is_last,
                        after_loads=flush_all if is_last else flush_store,
                        issue_next=make_issue_next(pos),
                    )
                elif dr_shared:
                    st = section_dr_shared(pre_w1[pos], after_loads=flush_store,
                                           issue_next=make_issue_next(pos))
                else:
                    st = section(sh_cap, pre_w1[pos], sw2[:], tsht[:], sout[:],
                                 swg_sb, sb1_sb, sb2_sb,
                                 after_loads=flush_store,
                                 issue_next=make_issue_next(pos))
                if st is not None:
                    pending.append(st)
            flush_all()

    nc.compile()
    return nc


def _get_program(caps, has_bias):
    key = (tuple(caps), PREC, has_bias)
    if key not in _PROG_CACHE:
        _PROG_CACHE[key] = _build_program(tuple(caps), has_bias)
    return _PROG_CACHE[key]


def _route(x, norm_w, Wr, bias):
    """Host-side norm + router + top-k (matches jax.lax.top_k tie-breaking)."""
    xf = x.reshape(T, D).astype(np.float32)
    ms = np.mean(xf * xf, axis=-1, keepdims=True, dtype=np.float32)
    t = (xf * (1.0 / np.sqrt(ms + EPS)) * norm_w).astype(np.float32)
    raw = t @ Wr.T
    aff = raw + bias
    idx = np.argsort(-aff, axis=-1, kind="stable")[:, :K]
    aff_k = np.take_along_axis(raw, idx, axis=1)
    w = aff_k / aff_k.sum(-1, keepdims=True)
    return t, idx.astype(np.int64), w.astype(np.float32)


def _gather_block(t, toks, cap):
    """tokens (cnt, D) -> [128, KD, cap] SBUF layout block (zero padded)."""
    blk = np.zeros((128, KD, cap), _np_tdt())
    g = t[toks].T.reshape(KD, 128, len(toks)).transpose(1, 0, 2)
    blk[:, :, : len(toks)] = g.astype(_np_tdt())
    return blk


def _decode_block(blk, cnt):
    """[128, KD, cap] device output block -> (cnt, D) token outputs."""
    cap = blk.shape[2]
    return blk.transpose(1, 0, 2).reshape(D, cap)[:, :cnt].T.astype(np.float32)


def kernel(**inputs):
    x = np.asarray(inputs["x"], dtype=np.float32)
    norm_w = np.asarray(inputs["norm_w"], dtype=np.float32)
    Wr = np.asarray(inputs["Wr"], dtype=np.float32)
    bias = np.asarray(inputs["bias"], dtype=np.float32)
    sW1 = np.asarray(inputs["sW1"], dtype=np.float32)
    sb1 = np.asarray(inputs["sb1"], dtype=np.float32)
    sW2 = np.asarray(inputs["sW2"], dtype=np.float32)
    sb2 = np.asarray(inputs["sb2"], dtype=np.float32)
    sWg = np.asarray(inputs["sWg"], dtype=np.float32)
    rW1 = np.asarray(inputs["rW1"], dtype=np.float32)
    rb1 = np.asarray(inputs["rb1"], dtype=np.float32)
    rW2 = np.asarray(inputs["rW2"], dtype=np.float32)
    rb2 = np.asarray(inputs["rb2"], dtype=np.float32)
    rWg = np.asarray(inputs["rWg"], dtype=np.float32)

    t, idx, w = _route(x, norm_w, Wr, bias)

    # per-expert token lists (token order ascending within each expert)
    flat_e = idx.ravel()
    flat_tok = np.repeat(np.arange(T), K)
    flat_w = w.ravel()
    order = np.argsort(flat_e, kind="stable")
    se, st, sw = flat_e[order], flat_tok[order], flat_w[order]
    counts = np.bincount(flat_e, minlength=E_R)
    bounds = np.concatenate([[0], np.cumsum(counts)])

    # split any over-512 expert into <=512 pieces (512 = max matmul free dim
    # for one PSUM bank at fp32)
    pieces = []  # (expert, tok_ids, weights)
    for e in range(E_R):
        lo, hi = bounds[e], bounds[e + 1]
        for s in range(lo, hi, 512):
            pieces.append((e, st[s : min(s + 512, hi)], sw[s : min(s + 512, hi)]))
    n_slots = max(N_SLOTS, -(-len(pieces) // 8))

    # snake assignment: sort pieces by size desc; rank-group of 8 -> one slot
    # index across all cores; within each group assign large->small to the
    # cores with the smallest running totals. Slot capacity = group max
    # rounded up to 16 (compile-time constant; identical inputs -> identical
    # caps -> NEFF cache hit).
    pieces.sort(key=lambda p: -len(p[1]))
    slot_of_core = [[None] * n_slots for _ in range(8)]
    totals = np.zeros(8, np.int64)
    caps = []
    for j in range(n_slots):
        group = pieces[j * 8 : (j + 1) * 8]
        core_order = np.argsort(totals, kind="stable")
        for gi, piece in enumerate(group):
            c = core_order[gi]
            slot_of_core[c][j] = piece
            totals[c] += len(piece[1])
        gmax = max((len(p[1]) for p in group), default=16)
        caps.append(min(512, max(32, -(-gmax // 16) * 16)))
    if PREC == "f32r":
        caps = [max(256, c) for c in caps]  # f32r needs N>=256 for full rate
    caps.append(SH_TOK)

    has_bias = bool(
        np.any(rb1) or np.any(rb2) or np.any(sb1) or np.any(sb2)
    )
    nc = _get_program(caps, has_bias)
    offs = np.concatenate([[0], np.cumsum(caps)]).astype(int)
    sumcap = int(offs[-2])

    wg_pre = _prearrange(rWg, KH)
    dr_shared = PREC == "fp8" and not has_bias
    if dr_shared:
        swg_pre = _e4cast(_prearrange(sWg, KH, raw=True), W4SC).reshape(128, KH, H)
        sw1_pre = [
            _e4cast(_prearrange_w1(sW1[e], raw=True), W4SC).reshape(128, KH * KD, 128)
            for e in range(E_S)
        ]
        sw2_pre = [
            _e4cast(_prearrange(sW2[e], KH, raw=True), W4SC).reshape(128, KH, D)
            for e in range(E_S)
        ]
    else:
        swg_pre = _prearrange(sWg, KH)
        sw1_pre = [_prearrange_w1(sW1[e]) for e in range(E_S)]
        sw2_pre = [_prearrange(sW2[e], KH) for e in range(E_S)]
    w1_pre = {}
    w2_pre = {}
    in_maps = []
    for c in range(8):
        xgt = np.zeros((128, KD * sumcap), _np_tdt())
        w1_stack = np.zeros((n_slots, 128, KD * H), _np_wdt())
        w2_stack = np.zeros((n_slots, 128, KH * D), _np_wdt())
        b1_arr = np.zeros((128, n_slots * 4), np.float32)
        b2_arr = np.zeros((128, n_slots * 8), np.float32)
        for j in range(n_slots):
            piece = slot_of_core[c][j]
            if piece is None:
                continue
            e, toks, _ = piece
            xgt[:, offs[j] * KD : offs[j + 1] * KD] = _gather_block(
                t, toks, caps[j]
            ).reshape(128, KD * caps[j])
            if e not in w1_pre:
                w1_pre[e] = _prearrange_w1(rW1[e])
                w2_pre[e] = _prearrange(rW2[e], KH)
            w1_stack[j] = w1_pre[e]
            w2_stack[j] = w2_pre[e]
            b1_arr[:, j * 4 : (j + 1) * 4] = rb1[e, 0].reshape(4, 128).T
            b2_arr[:, j * 8 : (j + 1) * 8] = rb2[e, 0].reshape(8, 128).T
        qc, se_ = c % 4, c // 4
        sh_toks = np.arange(qc * SH_TOK, (qc + 1) * SH_TOK)
        if dr_shared:
            tsh = _e4cast(
                t[sh_toks].T.reshape(KD, 128, SH_TOK).transpose(1, 0, 2), T4SC)
        else:
            tsh = _gather_block(t, sh_toks, SH_TOK).reshape(128, KD * SH_TOK)
        in_maps.append({
            "xgt": xgt,
            "w1s": w1_stack,
            "w2s": w2_stack,
            "b1s": b1_arr,
            "b2s": b2_arr,
            "wg": wg_pre,
            "swg": swg_pre,
            "tsht": tsh,
            "sw1": sw1_pre[se_],
            "sw2": sw2_pre[se_],
            "sb1": sb1[se_, 0].reshape(4, 128).T.copy(),
            "sb2": sb2[se_, 0].reshape(8, 128).T.copy(),
        })

    from concourse.bass_utils import run_bass_kernel_spmd

    global _LAST_IN_MAPS
    _LAST_IN_MAPS = in_maps
    res = run_bass_kernel_spmd(nc, in_maps, core_ids=list(range(8)))

    out = x.reshape(T, D).copy()
    for c in range(8):
        qc = c % 4
        so = res.results[c]["sout"].reshape(128, KD, SH_TOK)
        out[qc * SH_TOK : (qc + 1) * SH_TOK] += _decode_block(so, SH_TOK)
        ro = res.results[c]["rout"]
        for j in range(n_slots):
            piece = slot_of_core[c][j]
            if piece is None:
                continue
            _, toks, wv = piece
            blk = ro[:, offs[j] * KD : offs[j + 1] * KD].reshape(128, KD, caps[j])
            out[toks] += wv[:, None] * _decode_block(blk, len(toks))
    return out.reshape(B, S, D).astype(np.float32)



# revision 6
# speedup vs baseline: 1.2939x; 1.2939x over previous
"""DeepSeekMoE layer on 8 Trainium2 NeuronCores.

Strategy (expert-parallel):
  - Host: RMSNorm + router matmul + top-k + per-expert token gather
    (routing is tiny: 2048x1024 @ 1024x64). Tokens for each routed
    expert are gathered into per-expert slots and bin-packed onto the
    8 cores (rank-grouped so slot j holds similarly-sized experts on
    every core; per-slot capacities are compile-time constants rounded
    to 16).
  - Device (SPMD, one program on cores 0-7): for each expert slot,
    stream W1/W2 from HBM once and run the FFN on the gathered token
    batch entirely transposed ([D, tokens] layout) so every matmul
    contracts along partitions:
        h^T = W1^T x^T (+b1); g^T = silu(Wg^T h^T) * h^T;
        o^T = W2^T g^T (+b2)
    Default precision "fp8": e3m4 weights (stored x256, unscaled on the
    PSUM->SBUF copy) and e3m4 gathered tokens, bf16 intermediate
    activations, fp32 PSUM accumulate (~2e-3 final rel err, ~14MB HBM
    traffic per core). Weight loads issue on the sync HWDGE ring in
    0.5MB chunks (bigger chunks measurably steal SBUF ports from the
    PE), token loads on the scalar ring; output stores also ride the
    sync ring but are emitted two sections late so a store (which waits
    on compute) never head-of-line-blocks weight prefetch. Section 0
    loads W1 as 4 m-tiles and tokens as 8 k-tiles to cut the cold-start
    head; the final section stores per-chunk to cut the tail. PSUM
    tiles hold pairs of m-tiles so PSUM->SBUF moves are single fat DVE
    ops when biases are zero.
  - Host: scatter-add weighted expert outputs + shared + residual.

Self-contained: shapes hardcoded for B=2, S=1024, D=1024, H=512,
E_R=64, K=6, E_S=2.
"""

import numpy as np
from contextlib import ExitStack

B, S, D, H, E_R, K, E_S = 2, 1024, 1024, 512, 64, 6, 2
T = B * S
EPS = 1.1920929e-07

PREC = "fp8"         # "fp8" (e3m4 weights+tokens, ~2e-3 rel err),
                     # "bf16" (~3.6e-4), or "f32r" (~2e-5, 2x traffic)
OSCALE = 16.0        # dr_all: outputs stored as o*16 in e4m3
OCOPY = OSCALE / (4096.0 * 32.0)   # PSUM(131072*o) -> osb(16*o)
WSCALE = 256.0       # fp8: weights are stored as w*WSCALE in e3m4 (max |w|
INV_WSCALE = 1.0 / WSCALE  # ~1/32 -> 8.0, within e3m4's +-15.5 normal range)
# the shared 512-token section runs DoubleRow (2 fp8 weights/PE cell, ~1.7x)
# which requires e4m3 operands; power-of-2 scales place values in e4m3 range
W4SC = 4096.0        # shared weights stored as w*4096 (max ~128 < 240)
T4SC = 16.0          # shared tokens stored as x*16
H4SC = 32.0          # shared h/g intermediates stored as h*32
INV_H4 = H4SC / (W4SC * T4SC)   # PSUM -> h tile
INV_G4 = 1.0 / (W4SC * H4SC)    # PSUM -> silu input / output tile
N_SLOTS = 8          # routed expert slots per core
SH_TOK = 512         # shared-expert tokens per core
KD = D // 128        # 8 k-tiles for the D contraction
KH = H // 128        # 4 k-tiles for the H contraction

_PROG_CACHE = {}


def _np_wdt():
    import ml_dtypes
    if PREC == "bf16":
        return ml_dtypes.bfloat16
    if PREC == "fp8":
        return ml_dtypes.float8_e3m4
    return np.float32


def _np_adt():
    import ml_dtypes
    if PREC == "f32r":
        return np.float32
    return ml_dtypes.bfloat16


def _np_tdt():
    """Host dtype for token blocks (device moving operand of the W1 stage)."""
    import ml_dtypes
    if PREC == "fp8":
        return ml_dtypes.float8_e3m4
    return _np_adt()


def _wcast(w):
    """Cast a prearranged fp32 weight block to the device weight dtype,
    scaling into e3m4 range for fp8."""
    if PREC == "fp8":
        return np.clip(w * WSCALE, -15.5, 15.5).astype(_np_wdt())
    return w.astype(_np_wdt())


def _e4cast(w, scale):
    import ml_dtypes
    return np.clip(w.astype(np.float32) * scale, -240.0, 240.0).astype(
        ml_dtypes.float8_e4m3fn)


def _prearrange(w, ktiles, raw=False):
    """[K*128, N] -> [128, K*N] so each SBUF partition's row is one
    contiguous DRAM read."""
    n = w.shape[1]
    out = np.ascontiguousarray(
        w.reshape(ktiles, 128, n).transpose(1, 0, 2).reshape(128, ktiles * n)
    )
    return out if raw else _wcast(out)


def _prearrange_w1(w, raw=False):
    """[D, H] -> [128, KH*KD*128] m-major ([m][k][i] per partition) so W1 can
    load as 4 independent m-tiles."""
    out = np.ascontiguousarray(
        w.reshape(KD, 128, KH, 128).transpose(1, 2, 0, 3).reshape(128, KH * KD * 128)
    )
    return out if raw else _wcast(out)


def _build_program(caps, has_bias):
    """caps: per-section token capacities; the last entry is the shared
    512-token section, the rest are routed expert slots."""
    import concourse.tile as tile
    from concourse import bacc, mybir

    f32 = mybir.dt.float32
    if PREC == "fp8":
        wdt = mybir.dt.float8e3
    elif PREC == "f32r":
        wdt = mybir.dt.float32r
    else:
        wdt = mybir.dt.bfloat16
    adt = mybir.dt.float32r if PREC == "f32r" else mybir.dt.bfloat16
    tdt = mybir.dt.float8e3 if PREC == "fp8" else adt
    odt = f32 if PREC == "f32r" else mybir.dt.bfloat16
    inv = INV_WSCALE if PREC == "fp8" else 1.0
    AF = mybir.ActivationFunctionType
    ALU = mybir.AluOpType
    # the shared section uses DoubleRow e4m3 when fp8 and bias-free
    dr_shared = PREC == "fp8" and not has_bias
    e4 = mybir.dt.float8e4
    swdt = e4 if dr_shared else wdt
    stdt = e4 if dr_shared else tdt

    n_slots = len(caps) - 1
    offs = np.concatenate([[0], np.cumsum(caps)])
    sumcap = int(offs[-2])          # routed columns only
    sh_cap = caps[-1]

    nc = bacc.Bacc("TRN2", target_bir_lowering=False, debug=False)

    xgt = nc.dram_tensor("xgt", [128, KD * sumcap], tdt, kind="ExternalInput").ap()
    w1s = nc.dram_tensor("w1s", [n_slots, 128, KD * H], wdt, kind="ExternalInput").ap()
    w2s = nc.dram_tensor("w2s", [n_slots, 128, KH * D], wdt, kind="ExternalInput").ap()
    b1s = nc.dram_tensor("b1s", [128, n_slots * 4], f32, kind="ExternalInput").ap()
    b2s = nc.dram_tensor("b2s", [128, n_slots * 8], f32, kind="ExternalInput").ap()
    wg = nc.dram_tensor("wg", [128, KH * H], wdt, kind="ExternalInput").ap()
    if dr_shared:
        swg = nc.dram_tensor("swg", [128, KH, H], swdt, kind="ExternalInput").ap()
        tsht = nc.dram_tensor("tsht", [128, KD, sh_cap], stdt,
                              kind="ExternalInput").ap()
        sw1 = nc.dram_tensor("sw1", [128, KH * KD, 128], swdt,
                             kind="ExternalInput").ap()
        sw2 = nc.dram_tensor("sw2", [128, KH, D], swdt, kind="ExternalInput").ap()
    else:
        swg = nc.dram_tensor("swg", [128, KH * H], wdt, kind="ExternalInput").ap()
        tsht = nc.dram_tensor("tsht", [128, KD * sh_cap], tdt,
                              kind="ExternalInput").ap()
        sw1 = nc.dram_tensor("sw1", [128, KD * H], wdt, kind="ExternalInput").ap()
        sw2 = nc.dram_tensor("sw2", [128, KH * D], wdt, kind="ExternalInput").ap()
    sb1 = nc.dram_tensor("sb1", [128, 4], f32, kind="ExternalInput").ap()
    sb2 = nc.dram_tensor("sb2", [128, 8], f32, kind="ExternalInput").ap()
    rout = nc.dram_tensor("rout", [128, KD * sumcap], odt, kind="ExternalOutput").ap()
    sout = nc.dram_tensor("sout", [128, KD * sh_cap], odt, kind="ExternalOutput").ap()

    with tile.TileContext(nc) as tc:
        with ExitStack() as ctx:
            consts = ctx.enter_context(tc.tile_pool(name="consts", bufs=1))
            w1p = ctx.enter_context(tc.tile_pool(name="w1p", bufs=5))
            w2p = ctx.enter_context(tc.tile_pool(name="w2p", bufs=5))
            xgp = ctx.enter_context(tc.tile_pool(name="xgp", bufs=4))
            hp = ctx.enter_context(tc.tile_pool(name="hp", bufs=3))
            gpp = ctx.enter_context(tc.tile_pool(name="gpp", bufs=2))
            gp = ctx.enter_context(tc.tile_pool(name="gp", bufs=3))
            op = ctx.enter_context(tc.tile_pool(name="op", bufs=3))
            php = ctx.enter_context(tc.tile_pool(name="php", bufs=3, space="PSUM"))
            pgp = ctx.enter_context(tc.tile_pool(name="pgp", bufs=2, space="PSUM"))
            pop = ctx.enter_context(tc.tile_pool(name="pop", bufs=3, space="PSUM"))

            # biases ride the idle gpsimd ring so the sync ring's first
            # issue is section 0's W1; gate weights ride the scalar ring
            # right after the first token load (see after_xg below)
            b1_sb = consts.tile([128, n_slots * 4], f32, tag="b1")
            nc.gpsimd.dma_start(b1_sb[:], b1s[:])
            b2_sb = consts.tile([128, n_slots * 8], f32, tag="b2")
            nc.gpsimd.dma_start(b2_sb[:], b2s[:])
            sb1_sb = consts.tile([128, 4], f32, tag="sb1")
            nc.gpsimd.dma_start(sb1_sb[:], sb1[:])
            sb2_sb = consts.tile([128, 8], f32, tag="sb2")
            nc.gpsimd.dma_start(sb2_sb[:], sb2[:])
            wg_sb = consts.tile([128, KH * H], wdt, tag="wg")
            if dr_shared:
                swg_sb = consts.tile([128, KH, H], swdt, tag="swg")
            else:
                swg_sb = consts.tile([128, KH * H], wdt, tag="swg")

            # warm-up: dependency-free matmuls on (uninitialized) SBUF run
            # while the first weight DMAs stream in, releasing the PE HAM
            # clock throttle before real work arrives
            warm_src = consts.tile([128, 256], adt, tag="warmsrc", name="warmsrc")
            nc.vector.memset(warm_src[:], 0.0)
            warm_ps = php.tile([128, 256], f32, tag="ph", name="warmps")
            for _ in range(12):
                nc.tensor.matmul(
                    warm_ps[:], warm_src[:, 0:128], warm_src[:],
                    start=True, stop=True,
                )

            def section(cap, w1sb, w2_src, xg_src, out_dst, wgt, b1ap, b2ap,
                        after_xg=None, last=False, w1_tiles=None, xg_tiles=None,
                        after_loads=None, issue_next=None):
                """One expert FFN pass over `cap` tokens (transposed layout).
                W1 arrives preloaded (issued one section earlier on the sync
                ring, ahead of the previous section's W2, so the W1 stage
                never waits); issue_next emits the NEXT section's W1 load
                before this section's W2 load. PSUM tiles hold pairs of
                m-tiles (one bank) so PSUM->SBUF moves are single fat ops
                when biases are zero."""
                p2 = 2 if cap <= 256 else 1   # m-tiles per PSUM bank
                if xg_tiles is None:
                    xgsb = xgp.tile([128, KD * cap], tdt, tag="xg")
                    nc.scalar.dma_start(xgsb[:], xg_src)
                if after_xg is not None:
                    after_xg()
                if issue_next is not None:
                    issue_next()
                w2sb = w2p.tile([128, KH * D], wdt, tag="w2")
                nc.sync.dma_start(w2sb[:], w2_src)
                if after_loads is not None:
                    after_loads()

                def w1ap(m, k):
                    if w1_tiles is not None:
                        return w1_tiles[m][:, k * 128 : (k + 1) * 128]
                    return w1sb[:, m * KD * 128 + k * 128 : m * KD * 128 + (k + 1) * 128]

                def xgap(k):
                    if xg_tiles is not None:
                        return xg_tiles[k][:]
                    return xgsb[:, k * cap : (k + 1) * cap]

                hsb = hp.tile([128, KH * cap], adt, tag="h")
                for mp in range(KH // p2):
                    ph = php.tile([128, p2 * cap], f32, tag="ph")
                    for sub in range(p2):
                        m = p2 * mp + sub
                        for k in range(KD):
                            nc.tensor.matmul(
                                ph[:, sub * cap : (sub + 1) * cap],
                                w1ap(m, k),
                                xgap(k),
                                start=(k == 0),
                                stop=(k == KD - 1),
                            )
                    if has_bias:
                        for sub in range(p2):
                            m = p2 * mp + sub
                            nc.vector.tensor_scalar(
                                hsb[:, m * cap : (m + 1) * cap],
                                ph[:, sub * cap : (sub + 1) * cap],
                                inv, b1ap[:, m : m + 1],
                                ALU.mult, ALU.add,
                            )
                    elif inv != 1.0:
                        nc.vector.tensor_scalar_mul(
                            hsb[:, p2 * mp * cap : p2 * (mp + 1) * cap], ph[:], inv
                        )
                    else:
                        nc.vector.tensor_copy(
                            hsb[:, p2 * mp * cap : p2 * (mp + 1) * cap], ph[:]
                        )

                gsb = gp.tile([128, KH * cap], adt, tag="g")
                for mp in range(KH // p2):
                    pg = pgp.tile([128, p2 * cap], f32, tag="pg")
                    for sub in range(p2):
                        m = p2 * mp + sub
                        for k in range(KH):
                            nc.tensor.matmul(
                                pg[:, sub * cap : (sub + 1) * cap],
                                wgt[:, k * H + m * 128 : k * H + (m + 1) * 128],
                                hsb[:, k * cap : (k + 1) * cap],
                                start=(k == 0),
                                stop=(k == KH - 1),
                            )
                    gpre = gpp.tile([128, p2 * cap], adt, tag="gpre")
                    nc.scalar.activation(gpre[:], pg[:], AF.Silu, scale=inv)
                    nc.vector.tensor_mul(
                        gsb[:, p2 * mp * cap : p2 * (mp + 1) * cap],
                        gpre[:],
                        hsb[:, p2 * mp * cap : p2 * (mp + 1) * cap],
                    )

                osb = op.tile([128, KD * cap], odt, tag="o")
                for mp in range(KD // p2):
                    po = pop.tile([128, p2 * cap], f32, tag="po")
                    for sub in range(p2):
                        m = p2 * mp + sub
                        for k in range(KH):
                            nc.tensor.matmul(
                                po[:, sub * cap : (sub + 1) * cap],
                                w2sb[:, k * D + m * 128 : k * D + (m + 1) * 128],
                                gsb[:, k * cap : (k + 1) * cap],
                                start=(k == 0),
                                stop=(k == KH - 1),
                            )
                    if has_bias:
                        for sub in range(p2):
                            m = p2 * mp + sub
                            nc.vector.tensor_scalar(
                                osb[:, m * cap : (m + 1) * cap],
                                po[:, sub * cap : (sub + 1) * cap],
                                inv, b2ap[:, m : m + 1],
                                ALU.mult, ALU.add,
                            )
                    elif inv != 1.0:
                        nc.vector.tensor_scalar_mul(
                            osb[:, p2 * mp * cap : p2 * (mp + 1) * cap], po[:], inv
                        )
                    else:
                        nc.vector.tensor_copy(
                            osb[:, p2 * mp * cap : p2 * (mp + 1) * cap], po[:]
                        )
                    if last:
                        # final section: store chunks immediately so the
                        # kernel tail is one chunk, not 2MB
                        nc.sync.dma_start(
                            out_dst[:, p2 * mp * cap : p2 * (mp + 1) * cap],
                            osb[:, p2 * mp * cap : p2 * (mp + 1) * cap],
                        )
                if last:
                    return None
                def store():
                    nc.sync.dma_start(out_dst, osb[:])
                return store

            def section_dr_shared(w1sb, after_loads=None, issue_next=None):
                """Shared-expert section in DoubleRow mode: e4m3 operands,
                2 k-tiles per matmul at 2 fp8 weights per PE cell (~1.7x the
                bf16-rate stream at N=512). W1 arrives preloaded like the
                routed sections."""
                DR = mybir.MatmulPerfMode.DoubleRow
                cap = sh_cap
                xsb = xgp.tile([128, KD, cap], stdt, tag="xg4")
                nc.scalar.dma_start(xsb[:], tsht[:])
                if issue_next is not None:
                    issue_next()
                w2sb = w2p.tile([128, KH, D], swdt, tag="w24")
                nc.sync.dma_start(w2sb[:], sw2[:])
                if after_loads is not None:
                    after_loads()

                hsb = hp.tile([128, KH, cap], swdt, tag="h4")
                for m in range(KH):
                    ph = php.tile([128, cap], f32, tag="ph")
                    for kp in range(KD // 2):
                        nc.tensor.matmul(
                            ph[:],
                            w1sb[:, m * KD + 2 * kp : m * KD + 2 * kp + 2, :],
                            xsb[:, 2 * kp : 2 * kp + 2, :],
                            start=(kp == 0),
                            stop=(kp == KD // 2 - 1),
                            perf_mode=DR,
                        )
                    nc.vector.tensor_scalar_mul(hsb[:, m, :], ph[:], INV_H4)

                gsb = gp.tile([128, KH, cap], swdt, tag="g4")
                for m in range(KH):
                    pg = pgp.tile([128, cap], f32, tag="pg")
                    for kp in range(KH // 2):
                        nc.tensor.matmul(
                            pg[:],
                            swg_sb[:, 2 * kp : 2 * kp + 2, m * 128 : (m + 1) * 128],
                            hsb[:, 2 * kp : 2 * kp + 2, :],
                            start=(kp == 0),
                            stop=(kp == KH // 2 - 1),
                            perf_mode=DR,
                        )
                    gpre = gpp.tile([128, cap], adt, tag="gpre")
                    nc.scalar.activation(gpre[:], pg[:], AF.Silu, scale=INV_G4)
                    # hsb holds 32*h so the product lands at 32*g, already in
                    # e4m3 range for the W2 stage
                    nc.vector.tensor_mul(gsb[:, m, :], gpre[:], hsb[:, m, :])

                osb = op.tile([128, KD * cap], odt, tag="o")
                for m in range(KD):
                    po = pop.tile([128, cap], f32, tag="po")
                    for kp in range(KH // 2):
                        nc.tensor.matmul(
                            po[:],
                            w2sb[:, 2 * kp : 2 * kp + 2, m * 128 : (m + 1) * 128],
                            gsb[:, 2 * kp : 2 * kp + 2, :],
                            start=(kp == 0),
                            stop=(kp == KH // 2 - 1),
                            perf_mode=DR,
                        )
                    nc.vector.tensor_scalar_mul(
                        osb[:, m * cap : (m + 1) * cap], po[:], INV_G4
                    )

                def store():
                    nc.sync.dma_start(sout[:], osb[:])
                return store

            def load_wg():
                nc.scalar.dma_start(wg_sb[:], wg[:])

            def load_swg():
                nc.scalar.dma_start(swg_sb[:], swg[:])

            # section 0 fast start: W1 as 4 independent m-tiles and tokens as
            # 8 k-tiles so the first matmul waits for ~160KB, not 1MB
            c0 = caps[0]
            sec0_xg = [consts.tile([128, c0], tdt, tag=f"xh{k}", name=f"xh{k}") for k in range(KD)]
            nc.scalar.dma_start(sec0_xg[0][:], xgt[:, 0:c0])
            nc.scalar.dma_start(sec0_xg[1][:], xgt[:, c0 : 2 * c0])
            load_wg()
            for k in range(2, KD):
                nc.scalar.dma_start(sec0_xg[k][:], xgt[:, k * c0 : (k + 1) * c0])
            sec0_w1 = [consts.tile([128, KD * 128], wdt, tag=f"w1h{m}", name=f"w1h{m}") for m in range(KH)]
            for m in range(KH):
                nc.sync.dma_start(
                    sec0_w1[m][:], w1s[0][:, m * KD * 128 : (m + 1) * KD * 128]
                )

            pending = []

            def flush_store():
                if len(pending) >= 2:
                    pending.pop(0)()

            def flush_all():
                while pending:
                    pending.pop(0)()

            # the shared section is PE-heavy relative to its DMA bytes, so it
            # runs mid-stream as a prefetch catch-up window; the kernel ends
            # on a routed slot with eager per-chunk stores
            calls = [("r", j) for j in range(4)] + [("s", None)] + \
                    [("r", j) for j in range(4, n_slots)]

            def w1_loader(pos):
                kind, j = calls[pos]
                if kind == "s" and dr_shared:
                    tl = w1p.tile([128, KH * KD, 128], swdt, tag="w14")
                    nc.sync.dma_start(tl[:], sw1[:])
                elif kind == "s":
                    tl = w1p.tile([128, KD * H], wdt, tag="w1")
                    nc.sync.dma_start(tl[:], sw1[:])
                else:
                    tl = w1p.tile([128, KD * H], wdt, tag="w1")
                    nc.sync.dma_start(tl[:], w1s[j])
                return tl

            pre_w1 = [None] * len(calls)
            pre_w1[1] = w1_loader(1)   # behind sec0's m-tiles, ahead of W2(0)

            def make_issue_next(pos):
                def f():
                    if pos + 1 < len(calls) and pre_w1[pos + 1] is None:
                        pre_w1[pos + 1] = w1_loader(pos + 1)
                return f

            for pos, (kind, j) in enumerate(calls):
                is_last = pos == len(calls) - 1
                if kind == "r":
                    lo, hi = int(offs[j]) * KD, int(offs[j + 1]) * KD
                    st = section(
                        caps[j],
                        pre_w1[pos],
                        w2s[j],
                        xgt[:, lo:hi],
                        rout[:, lo:hi],
                        wg_sb,
                        b1_sb[:, j * 4 : (j + 1) * 4],
                        b2_sb[:, j * 8 : (j + 1) * 8],
                        after_xg=load_swg if j == 1 else None,
                        w1_tiles=sec0_w1 if j == 0 else None,
                        xg_tiles=sec0_xg if j == 0 else None,
                        last=is_last,
                        after_loads=flush_all if is_last else flush_store,
                        issue_next=make_issue_next(pos),
                    )
                elif dr_shared:
                    st = section_dr_shared(pre_w1[pos], after_loads=flush_store,
                                           issue_next=make_issue_next(pos))
                else:
                    st = section(sh_cap, pre_w1[pos], sw2[:], tsht[:], sout[:],
                                 swg_sb, sb1_sb, sb2_sb,
                                 after_loads=flush_store,
                                 issue_next=make_issue_next(pos))
                if st is not None:
                    pending.append(st)
            flush_all()

    nc.compile()
    return nc


def _build_program_dr_all(caps):
    """All-DoubleRow e4m3 program (zero-bias fast path).

    Every section (8 routed slots + the shared 512-token section) runs the
    3-stage FFN with DoubleRow fp8 matmuls (2 k-tiles per op; measured on
    hw: per-op spacing = N*0.42ns at any N>=128, LDWEIGHTS fully hidden),
    i.e. 2x the bf16/e3m4 single-row rate. Scales: weights x4096, tokens
    x16, h x32, outputs x16 (all power-of-2, in e4m3 range).

    Engine budget (per core, measured sustained rates):
      PE ~33us <- critical path; vector (h-copies + gate mult + 1/4
      o-copies) ~28us; scalar (silu + 3/4 o-copies) ~27us.
    Rings: sync = W1 loads + output stores (stores issued two sections
    late); gpsimd = W2 loads; gpsimd = token loads + W2 loads; gate weights ride
    sync behind the section-0 W1 m-slices.
    """
    import concourse.tile as tile
    from concourse import bacc, mybir

    f32 = mybir.dt.float32
    e4 = mybir.dt.float8e4
    bf = mybir.dt.bfloat16
    AF = mybir.ActivationFunctionType
    DR = mybir.MatmulPerfMode.DoubleRow

    n_slots = len(caps) - 1
    offs = np.concatenate([[0], np.cumsum(caps)])
    sh_cap = caps[-1]

    nc = bacc.Bacc("TRN2", target_bir_lowering=False, debug=False)

    xg_t = [
        nc.dram_tensor(f"xg{j}", [128, KD, caps[j]], e4, kind="ExternalInput").ap()
        for j in range(n_slots)
    ]
    w1s = nc.dram_tensor("w1s", [n_slots, 128, KH * KD, 128], e4,
                         kind="ExternalInput").ap()
    w2s = nc.dram_tensor("w2s", [n_slots, 128, KH, D], e4,
                         kind="ExternalInput").ap()
    wg = nc.dram_tensor("wg", [128, KH, H], e4, kind="ExternalInput").ap()
    swg = nc.dram_tensor("swg", [128, KH, H], e4, kind="ExternalInput").ap()
    tsht = nc.dram_tensor("tsht", [128, KD, sh_cap], e4, kind="ExternalInput").ap()
    sw1 = nc.dram_tensor("sw1", [128, KH * KD, 128], e4, kind="ExternalInput").ap()
    sw2 = nc.dram_tensor("sw2", [128, KH, D], e4, kind="ExternalInput").ap()
    rout_t = [
        nc.dram_tensor(f"rout{j}", [128, KD, caps[j]], e4, kind="ExternalOutput").ap()
        for j in range(n_slots)
    ]
    sout = nc.dram_tensor("sout", [128, KD, sh_cap], e4, kind="ExternalOutput").ap()

    with tile.TileContext(nc) as tc:
        with ExitStack() as ctx:
            consts = ctx.enter_context(tc.tile_pool(name="consts", bufs=1))
            w1p = ctx.enter_context(tc.tile_pool(name="w1p", bufs=3))
            w2p = ctx.enter_context(tc.tile_pool(name="w2p", bufs=3))
            xgp = ctx.enter_context(tc.tile_pool(name="xgp", bufs=3))
            hp = ctx.enter_context(tc.tile_pool(name="hp", bufs=3))
            gpp = ctx.enter_context(tc.tile_pool(name="gpp", bufs=2))
            gp = ctx.enter_context(tc.tile_pool(name="gp", bufs=3))
            op = ctx.enter_context(tc.tile_pool(name="op", bufs=3))
            php = ctx.enter_context(tc.tile_pool(name="php", bufs=3, space="PSUM"))
            pgp = ctx.enter_context(tc.tile_pool(name="pgp", bufs=2, space="PSUM"))
            pop = ctx.enter_context(tc.tile_pool(name="pop", bufs=3, space="PSUM"))

            wg_sb = consts.tile([128, KH, H], e4, tag="wg")
            swg_sb = consts.tile([128, KH, H], e4, tag="swg")

            # warm-up: dependency-free matmuls release the PE HAM throttle
            # while the first loads stream in; they are first in the tensor
            # stream so the token-load descriptor writes below don't delay
            # them
            warm_src = consts.tile([128, 256], bf, tag="warmsrc", name="warmsrc")
            nc.vector.memset(warm_src[:], 0.0)
            warm_ps = php.tile([128, 256], f32, tag="ph", name="warmps")
            for _ in range(12):
                nc.tensor.matmul(
                    warm_ps[:], warm_src[:, 0:128], warm_src[:],
                    start=True, stop=True,
                )

            calls = [("r", j) for j in range(4)] + [("s", None)] + \
                    [("r", j) for j in range(4, n_slots)]

            def cap_of(pos):
                kind, j = calls[pos]
                return sh_cap if kind == "s" else caps[j]

            def w1_loader(pos):
                kind, j = calls[pos]
                tl = w1p.tile([128, KH * KD, 128], e4, tag="w1")
                nc.sync.dma_start(tl[:], sw1[:] if kind == "s" else w1s[j])
                return tl

            def w2_loader(pos):
                kind, j = calls[pos]
                tl = w2p.tile([128, KH, D], e4, tag="w2")
                nc.gpsimd.dma_start(tl[:], sw2[:] if kind == "s" else w2s[j])
                return tl

            def xg_loader(pos):
                kind, j = calls[pos]
                tl = xgp.tile([128, KD, cap_of(pos)], e4, tag="xg")
                nc.gpsimd.dma_start(tl[:], tsht[:] if kind == "s" else xg_t[j][:])
                return tl

            def section(cap, w1sb, w2sb, xsb, out_dst, wgt, last=False,
                        w1_tiles=None, xg_tiles=None, after_s1=None, o_vec=1):
                p2 = 2 if cap <= 256 else 1
                KP = KD // 2

                def w1ap(m, kp):
                    if w1_tiles is not None:
                        return w1_tiles[m][:, 2 * kp : 2 * kp + 2, :]
                    return w1sb[:, m * KD + 2 * kp : m * KD + 2 * kp + 2, :]

                def xgap(kp):
                    if xg_tiles is not None:
                        return xg_tiles[kp][:]
                    return xsb[:, 2 * kp : 2 * kp + 2, :]

                hsb = hp.tile([128, KH, cap], e4, tag="h")
                for mp in range(KH // p2):
                    ph = php.tile([128, p2 * cap], f32, tag="ph")
                    for sub in range(p2):
                        m = mp * p2 + sub
                        for kp in range(KP):
                            nc.tensor.matmul(
                                ph[:, sub * cap : (sub + 1) * cap],
                                w1ap(m, kp), xgap(kp),
                                start=(kp == 0), stop=(kp == KP - 1),
                                perf_mode=DR,
                            )
                    nc.vector.tensor_scalar_mul(
                        hsb[:, mp * p2 : (mp + 1) * p2, :], ph[:], INV_H4
                    )
                if after_s1 is not None:
                    after_s1()

                gsb = gp.tile([128, KH, cap], e4, tag="g")
                for mp in range(KH // p2):
                    pg = pgp.tile([128, p2 * cap], f32, tag="pg")
                    for sub in range(p2):
                        m = mp * p2 + sub
                        for kp in range(KH // 2):
                            nc.tensor.matmul(
                                pg[:, sub * cap : (sub + 1) * cap],
                                wgt[:, 2 * kp : 2 * kp + 2, m * 128 : (m + 1) * 128],
                                hsb[:, 2 * kp : 2 * kp + 2, :],
                                start=(kp == 0), stop=(kp == KH // 2 - 1),
                                perf_mode=DR,
                            )
                    gpre = gpp.tile([128, p2 * cap], e4, tag="gpre")
                    nc.scalar.activation(gpre[:], pg[:], AF.Silu, scale=INV_G4)
                    nc.vector.tensor_mul(
                        gsb[:, mp * p2 : (mp + 1) * p2, :],
                        gpre[:],
                        hsb[:, mp * p2 : (mp + 1) * p2, :],
                    )

                osb = op.tile([128, KD, cap], e4, tag="o")
                for mp in range(KD // p2):
                    po = pop.tile([128, p2 * cap], f32, tag="po")
                    for sub in range(p2):
                        m = mp * p2 + sub
                        for kp in range(KH // 2):
                            nc.tensor.matmul(
                                po[:, sub * cap : (sub + 1) * cap],
                                w2sb[:, 2 * kp : 2 * kp + 2, m * 128 : (m + 1) * 128],
                                gsb[:, 2 * kp : 2 * kp + 2, :],
                                start=(kp == 0), stop=(kp == KH // 2 - 1),
                                perf_mode=DR,
                            )
                    dst = osb[:, mp * p2 : (mp + 1) * p2, :]
                    if mp < o_vec:
                        nc.vector.tensor_scalar_mul(dst, po[:], OCOPY)
                    else:
                        nc.scalar.activation(dst, po[:], AF.Copy, scale=OCOPY)
                    if last:
                        nc.sync.dma_start(
                            out_dst[:, mp * p2 : (mp + 1) * p2, :], dst
                        )
                if last:
                    return None

                def store():
                    nc.sync.dma_start(out_dst[:], osb[:])
                return store

            # section 0 fast start: tokens as 4 k-pair tiles, W1 as 4 m-slices
            c0 = caps[0]
            sec0_xg = [
                consts.tile([128, 2, c0], e4, tag=f"xh{k}", name=f"xh{k}")
                for k in range(KD // 2)
            ]
            for k in range(KD // 2):
                nc.gpsimd.dma_start(sec0_xg[k][:], xg_t[0][:, 2 * k : 2 * k + 2, :])
            sec0_w1 = [
                consts.tile([128, KD, 128], e4, tag=f"w1h{m}", name=f"w1h{m}")
                for m in range(KH)
            ]
            for m in range(KH):
                nc.sync.dma_start(sec0_w1[m][:], w1s[0][:, m * KD : (m + 1) * KD, :])
            nc.sync.dma_start(wg_sb[:], wg[:])

            pre_w1 = [None] * len(calls)
            pre_w2 = [None] * len(calls)
            pre_xg = [None] * len(calls)
            pre_w2[0] = w2_loader(0)
            pre_w1[1] = w1_loader(1)
            nc.sync.dma_start(swg_sb[:], swg[:])
            pre_xg[1] = xg_loader(1)
            pre_w2[1] = w2_loader(1)

            pending = []

            def flush_store(n_keep):
                while len(pending) > n_keep:
                    pending.pop(0)()

            def make_issue(pos, is_last):
                def f():
                    if pos + 1 < len(calls):
                        if pre_xg[pos + 1] is None:
                            pre_xg[pos + 1] = xg_loader(pos + 1)
                        if pre_w1[pos + 1] is None:
                            pre_w1[pos + 1] = w1_loader(pos + 1)
                        if pre_w2[pos + 1] is None:
                            pre_w2[pos + 1] = w2_loader(pos + 1)
                    flush_store(0 if is_last else 2)
                return f

            for pos, (kind, j) in enumerate(calls):
                is_last = pos == len(calls) - 1
                if kind == "s":
                    out_dst = sout
                    wgt = swg_sb
                else:
                    out_dst = rout_t[j]
                    wgt = wg_sb
                st = section(
                    cap_of(pos), pre_w1[pos], pre_w2[pos], pre_xg[pos],
                    out_dst, wgt, last=is_last,
                    w1_tiles=sec0_w1 if pos == 0 else None,
                    xg_tiles=sec0_xg if pos == 0 else None,
                    after_s1=make_issue(pos, is_last),
                    o_vec=2 if kind == "s" else 1,
                )
                if st is not None:
                    pending.append(st)
            flush_store(0)

    nc.compile()
    return nc


def _get_program(caps, has_bias):
    key = (tuple(caps), PREC, has_bias)
    if key not in _PROG_CACHE:
        if PREC == "fp8" and not has_bias:
            _PROG_CACHE[key] = _build_program_dr_all(tuple(caps))
        else:
            _PROG_CACHE[key] = _build_program(tuple(caps), has_bias)
    return _PROG_CACHE[key]


def _route(x, norm_w, Wr, bias):
    """Host-side norm + router + top-k (matches jax.lax.top_k tie-breaking)."""
    xf = x.reshape(T, D).astype(np.float32)
    ms = np.mean(xf * xf, axis=-1, keepdims=True, dtype=np.float32)
    t = (xf * (1.0 / np.sqrt(ms + EPS)) * norm_w).astype(np.float32)
    raw = t @ Wr.T
    aff = raw + bias
    idx = np.argsort(-aff, axis=-1, kind="stable")[:, :K]
    aff_k = np.take_along_axis(raw, idx, axis=1)
    w = aff_k / aff_k.sum(-1, keepdims=True)
    return t, idx.astype(np.int64), w.astype(np.float32)


def _gather_block(t, toks, cap):
    """tokens (cnt, D) -> [128, KD, cap] SBUF layout block (zero padded)."""
    blk = np.zeros((128, KD, cap), _np_tdt())
    g = t[toks].T.reshape(KD, 128, len(toks)).transpose(1, 0, 2)
    blk[:, :, : len(toks)] = g.astype(_np_tdt())
    return blk


def _decode_block(blk, cnt):
    """[128, KD, cap] device output block -> (cnt, D) token outputs."""
    cap = blk.shape[2]
    return blk.transpose(1, 0, 2).reshape(D, cap)[:, :cnt].T.astype(np.float32)


def _gather_block_e4(t, toks, cap):
    """tokens (cnt, D) -> [128, KD, cap] e4m3 block scaled x16 (zero padded)."""
    import ml_dtypes
    blk = np.zeros((128, KD, cap), ml_dtypes.float8_e4m3fn)
    g = t[toks].T.reshape(KD, 128, len(toks)).transpose(1, 0, 2)
    blk[:, :, : len(toks)] = np.clip(g * T4SC, -240.0, 240.0).astype(
        ml_dtypes.float8_e4m3fn)
    return blk


def _run_dr_all(nc, x, t, caps, offs, n_slots, slot_of_core,
                rW1, rW2, rWg, sW1, sW2, sWg):
    """Host pack / run / combine for the all-DoubleRow program."""
    wg_pre = _e4cast(_prearrange(rWg, KH, raw=True), W4SC).reshape(128, KH, H)
    swg_pre = _e4cast(_prearrange(sWg, KH, raw=True), W4SC).reshape(128, KH, H)
    sw1_pre = [
        _e4cast(_prearrange_w1(sW1[e], raw=True), W4SC).reshape(128, KH * KD, 128)
        for e in range(E_S)
    ]
    sw2_pre = [
        _e4cast(_prearrange(sW2[e], KH, raw=True), W4SC).reshape(128, KH, D)
        for e in range(E_S)
    ]
    w1_pre = {}
    w2_pre = {}
    in_maps = []
    import ml_dtypes
    for c in range(8):
        w1_stack = np.zeros((n_slots, 128, KH * KD, 128), ml_dtypes.float8_e4m3fn)
        w2_stack = np.zeros((n_slots, 128, KH, D), ml_dtypes.float8_e4m3fn)
        im = {"w1s": w1_stack, "w2s": w2_stack, "wg": wg_pre, "swg": swg_pre}
        for j in range(n_slots):
            piece = slot_of_core[c][j]
            if piece is None:
                im[f"xg{j}"] = np.zeros((128, KD, caps[j]), ml_dtypes.float8_e4m3fn)
                continue
            e, toks, _ = piece
            im[f"xg{j}"] = _gather_block_e4(t, toks, caps[j])
            if e not in w1_pre:
                w1_pre[e] = _e4cast(
                    _prearrange_w1(rW1[e], raw=True), W4SC
                ).reshape(128, KH * KD, 128)
                w2_pre[e] = _e4cast(
                    _prearrange(rW2[e], KH, raw=True), W4SC
                ).reshape(128, KH, D)
            w1_stack[j] = w1_pre[e]
            w2_stack[j] = w2_pre[e]
        qc, se_ = c % 4, c // 4
        sh_toks = np.arange(qc * SH_TOK, (qc + 1) * SH_TOK)
        im["tsht"] = _gather_block_e4(t, sh_toks, SH_TOK)
        im["sw1"] = sw1_pre[se_]
        im["sw2"] = sw2_pre[se_]
        in_maps.append(im)

    from concourse.bass_utils import run_bass_kernel_spmd

    global _LAST_IN_MAPS
    _LAST_IN_MAPS = in_maps
    res = run_bass_kernel_spmd(nc, in_maps, core_ids=list(range(8)))

    inv_o = 1.0 / OSCALE
    out = x.reshape(T, D).copy()
    for c in range(8):
        qc = c % 4
        so = res.results[c]["sout"].astype(np.float32) * inv_o
        out[qc * SH_TOK : (qc + 1) * SH_TOK] += _decode_block(so, SH_TOK)
        for j in range(n_slots):
            piece = slot_of_core[c][j]
            if piece is None:
                continue
            _, toks, wv = piece
            blk = res.results[c][f"rout{j}"].astype(np.float32) * inv_o
            out[toks] += wv[:, None] * _decode_block(blk, len(toks))
    return out.reshape(B, S, D).astype(np.float32)


def kernel(**inputs):
    x = np.asarray(inputs["x"], dtype=np.float32)
    norm_w = np.asarray(inputs["norm_w"], dtype=np.float32)
    Wr = np.asarray(inputs["Wr"], dtype=np.float32)
    bias = np.asarray(inputs["bias"], dtype=np.float32)
    sW1 = np.asarray(inputs["sW1"], dtype=np.float32)
    sb1 = np.asarray(inputs["sb1"], dtype=np.float32)
    sW2 = np.asarray(inputs["sW2"], dtype=np.float32)
    sb2 = np.asarray(inputs["sb2"], dtype=np.float32)
    sWg = np.asarray(inputs["sWg"], dtype=np.float32)
    rW1 = np.asarray(inputs["rW1"], dtype=np.float32)
    rb1 = np.asarray(inputs["rb1"], dtype=np.float32)
    rW2 = np.asarray(inputs["rW2"], dtype=np.float32)
    rb2 = np.asarray(inputs["rb2"], dtype=np.float32)
    rWg = np.asarray(inputs["rWg"], dtype=np.float32)

    t, idx, w = _route(x, norm_w, Wr, bias)

    # per-expert token lists (token order ascending within each expert)
    flat_e = idx.ravel()
    flat_tok = np.repeat(np.arange(T), K)
    flat_w = w.ravel()
    order = np.argsort(flat_e, kind="stable")
    se, st, sw = flat_e[order], flat_tok[order], flat_w[order]
    counts = np.bincount(flat_e, minlength=E_R)
    bounds = np.concatenate([[0], np.cumsum(counts)])

    # split any over-512 expert into <=512 pieces (512 = max matmul free dim
    # for one PSUM bank at fp32)
    pieces = []  # (expert, tok_ids, weights)
    for e in range(E_R):
        lo, hi = bounds[e], bounds[e + 1]
        for s in range(lo, hi, 512):
            pieces.append((e, st[s : min(s + 512, hi)], sw[s : min(s + 512, hi)]))
    n_slots = max(N_SLOTS, -(-len(pieces) // 8))

    # snake assignment: sort pieces by size desc; rank-group of 8 -> one slot
    # index across all cores; within each group assign large->small to the
    # cores with the smallest running totals. Slot capacity = group max
    # rounded up to 16 (compile-time constant; identical inputs -> identical
    # caps -> NEFF cache hit).
    pieces.sort(key=lambda p: -len(p[1]))
    slot_of_core = [[None] * n_slots for _ in range(8)]
    totals = np.zeros(8, np.int64)
    caps = []
    for j in range(n_slots):
        group = pieces[j * 8 : (j + 1) * 8]
        core_order = np.argsort(totals, kind="stable")
        for gi, piece in enumerate(group):
            c = core_order[gi]
            slot_of_core[c][j] = piece
            totals[c] += len(piece[1])
        gmax = max((len(p[1]) for p in group), default=16)
        caps.append(min(512, max(32, -(-gmax // 16) * 16)))
    if PREC == "f32r":
        caps = [max(256, c) for c in caps]  # f32r needs N>=256 for full rate
    caps.append(SH_TOK)

    has_bias = bool(
        np.any(rb1) or np.any(rb2) or np.any(sb1) or np.any(sb2)
    )
    nc = _get_program(caps, has_bias)
    offs = np.concatenate([[0], np.cumsum(caps)]).astype(int)
    sumcap = int(offs[-2])

    dr_all = PREC == "fp8" and not has_bias
    if dr_all:
        return _run_dr_all(nc, x, t, caps, offs, n_slots, slot_of_core,
                           rW1, rW2, rWg, sW1, sW2, sWg)

    wg_pre = _prearrange(rWg, KH)
    dr_shared = PREC == "fp8" and not has_bias
    if dr_shared:
        swg_pre = _e4cast(_prearrange(sWg, KH, raw=True), W4SC).reshape(128, KH, H)
        sw1_pre = [
            _e4cast(_prearrange_w1(sW1[e], raw=True), W4SC).reshape(128, KH * KD, 128)
            for e in range(E_S)
        ]
        sw2_pre = [
            _e4cast(_prearrange(sW2[e], KH, raw=True), W4SC).reshape(128, KH, D)
            for e in range(E_S)
        ]
    else:
        swg_pre = _prearrange(sWg, KH)
        sw1_pre = [_prearrange_w1(sW1[e]) for e in range(E_S)]
        sw2_pre = [_prearrange(sW2[e], KH) for e in range(E_S)]
    w1_pre = {}
    w2_pre = {}
    in_maps = []
    for c in range(8):
        xgt = np.zeros((128, KD * sumcap), _np_tdt())
        w1_stack = np.zeros((n_slots, 128, KD * H), _np_wdt())
        w2_stack = np.zeros((n_slots, 128, KH * D), _np_wdt())
        b1_arr = np.zeros((128, n_slots * 4), np.float32)
        b2_arr = np.zeros((128, n_slots * 8), np.float32)
        for j in range(n_slots):
            piece = slot_of_core[c][j]
            if piece is None:
                continue
            e, toks, _ = piece
            xgt[:, offs[j] * KD : offs[j + 1] * KD] = _gather_block(
                t, toks, caps[j]
            ).reshape(128, KD * caps[j])
            if e not in w1_pre:
                w1_pre[e] = _prearrange_w1(rW1[e])
                w2_pre[e] = _prearrange(rW2[e], KH)
            w1_stack[j] = w1_pre[e]
            w2_stack[j] = w2_pre[e]
            b1_arr[:, j * 4 : (j + 1) * 4] = rb1[e, 0].reshape(4, 128).T
            b2_arr[:, j * 8 : (j + 1) * 8] = rb2[e, 0].reshape(8, 128).T
        qc, se_ = c % 4, c // 4
        sh_toks = np.arange(qc * SH_TOK, (qc + 1) * SH_TOK)
        if dr_shared:
            tsh = _e4cast(
                t[sh_toks].T.reshape(KD, 128, SH_TOK).transpose(1, 0, 2), T4SC)
        else:
            tsh = _gather_block(t, sh_toks, SH_TOK).reshape(128, KD * SH_TOK)
        in_maps.append({
            "xgt": xgt,
            "w1s": w1_stack,
            "w2s": w2_stack,
            "b1s": b1_arr,
            "b2s": b2_arr,
            "wg": wg_pre,
            "swg": swg_pre,
            "tsht": tsh,
            "sw1": sw1_pre[se_],
            "sw2": sw2_pre[se_],
            "sb1": sb1[se_, 0].reshape(4, 128).T.copy(),
            "sb2": sb2[se_, 0].reshape(8, 128).T.copy(),
        })

    from concourse.bass_utils import run_bass_kernel_spmd

    global _LAST_IN_MAPS
    _LAST_IN_MAPS = in_maps
    res = run_bass_kernel_spmd(nc, in_maps, core_ids=list(range(8)))

    out = x.reshape(T, D).copy()
    for c in range(8):
        qc = c % 4
        so = res.results[c]["sout"].reshape(128, KD, SH_TOK)
        out[qc * SH_TOK : (qc + 1) * SH_TOK] += _decode_block(so, SH_TOK)
        ro = res.results[c]["rout"]
        for j in range(n_slots):
            piece = slot_of_core[c][j]
            if piece is None:
                continue
            _, toks, wv = piece
            blk = ro[:, offs[j] * KD : offs[j + 1] * KD].reshape(128, KD, caps[j])
            out[toks] += wv[:, None] * _decode_block(blk, len(toks))
    return out.reshape(B, S, D).astype(np.float32)

